# revision 73
# baseline (speedup 1.0000x reference)
"""Multi-head causal attention (dense transformer block) on 8 TRN2 NeuronCores.

Problem: B=2, S=2048, D_MODEL=768, H=12 heads, D_HEAD=64, fp32 I/O.

Sharding: 24 (batch, head) units over 8 cores -> 3 heads x 1 batch per core.
Cores 0-3 handle batch 0 (heads 0-2, 3-5, 6-8, 9-11), cores 4-7 batch 1.
Each core computes its heads' contribution to out[b] = sum_h z_h @ W_O[h];
the host sums the 4 bf16 partials per batch in f32 and adds b_O.

Per-core dataflow (all matmuls bf16 -> fp32 PSUM):
  - x[b]^T staged in SBUF as 6 chunks [128, 2048].
  - QT/KT in "head-transposed" layout [64, S] packed 2-per-tile:
      QT01 = [Q_h0*s | Q_h1*s]^T x  (scale 1/sqrt(64) folded into W_Q/b_Q)
      KT01 = [K_h0 | K_h1]^T x, QKT2 = [Q_h2*s | K_h2]^T x
    plus QK2swap = partition-swap of QKT2 so h2 has Q and K on BOTH
    partition halves.
  - scores TRANSPOSED: sT[k, q] = KT^T-slice x QT (K=64 contraction).
    The two K=64 matmuls of a (h0, h1) pair are emitted adjacently with
    disjoint row-groups (partitions 0-63 / 64-127) so the PE runs them
    CONCURRENTLY via tile_position row packing; h2 packs its two j-tiles
    the same way using QK2swap.
  - h0/h1 rounds and h2 rounds are separate streams: pair rounds only need
    Q,K (projected first), so ScalarE exp work starts early; h2 rounds and
    PV lag until QK2/V land without stalling ACT.
  - exp via ScalarE (one instr per [128, 1024] j-pair), causal mask via
    memset + one triangular-mask multiply per diagonal 128x128 tile.
  - zT[h] = sum_j [V_h[j] | 1]^T x PT[j]  (unnormalized; the ones column
    makes output row 64 the softmax denominator), normalized as
    zT * broadcast(1/sums) when copying PSUM->SBUF.
  - output projection per q-block as soon as its zT is normalized:
    out[q, :] = zT01^T-slice @ WO01 + zT2^T-slice @ WO2, copied to SBUF as
    bf16 and DMA'd out immediately (halves the output-DMA tail).
"""

import numpy as np
import ml_dtypes
from contextlib import ExitStack

import concourse.bass as bass
import concourse.mybir as mybir
import concourse.tile as tile
from concourse import bacc
from concourse.bass_utils import run_bass_kernel_spmd

BF16 = mybir.dt.bfloat16
F32 = mybir.dt.float32
AF = mybir.ActivationFunctionType
NPBF16 = ml_dtypes.bfloat16

B, S, D, H, DH = 2, 2048, 768, 12, 64
N_CORES = 8
DCH = D // 128          # 6 d_model chunks
NKT = S // 128          # 16 k tiles
QB = 512                # q block width
NQB = S // QB           # 4 q blocks
G = 2                   # k-tiles per exp group

TRACE_ENABLED = False
LAST_EXEC_NS = None
LAST_RESULT = None
_BUILT = None


def build_nc():
    nc = bacc.Bacc("TRN2", target_bir_lowering=False, debug=False)

    xT_d = nc.dram_tensor("xT", [D, S], BF16, kind="ExternalInput")
    wq01_d = nc.dram_tensor("wq01", [D, 128], BF16, kind="ExternalInput")
    wk01_d = nc.dram_tensor("wk01", [D, 128], BF16, kind="ExternalInput")
    wqk2_d = nc.dram_tensor("wqk2", [D, 128], BF16, kind="ExternalInput")
    wv_d = nc.dram_tensor("wv", [D, 195], BF16, kind="ExternalInput")
    wo01_d = nc.dram_tensor("wo01", [128, D], BF16, kind="ExternalInput")
    wo2_d = nc.dram_tensor("wo2", [64, D], BF16, kind="ExternalInput")
    bq01_d = nc.dram_tensor("bq01", [128, 1], F32, kind="ExternalInput")
    bk01_d = nc.dram_tensor("bk01", [128, 1], F32, kind="ExternalInput")
    bqk2_d = nc.dram_tensor("bqk2", [128, 1], F32, kind="ExternalInput")
    bv_d = nc.dram_tensor("bv", [128, 195], F32, kind="ExternalInput")
    out_d = nc.dram_tensor("out_p", [S, D], BF16, kind="ExternalOutput")

    tri_np = np.triu(np.ones((128, 128), np.float32)).astype(NPBF16)
    tri_d = nc.inline_tensor(tri_np, "tri")

    with tile.TileContext(nc) as tc, ExitStack() as ctx:
        persist = ctx.enter_context(tc.tile_pool(name="persist", bufs=1))

        # ---- HAM warm-up: ~5us of dummy matmuls spanning the input-DMA
        # wait, so the PE clock is at 2.4GHz when the first projection
        # matmuls issue ----
        with tc.tile_pool(name="warm_ps", bufs=1, space="PSUM") as warm_pool:
            wz = persist.tile([128, 128], BF16, tag="wz")
            nc.vector.memset(wz[:], 0.0)
            wps = warm_pool.tile([128, 128], F32, tag="warm")
            for _ in range(40):
                nc.tensor.matmul(wps[:], wz[:], wz[:], start=True, stop=True)

        # ---- stage inputs in SBUF. DMA issue order feeds the d-outer
        # Q+K projection: wq01, xt0, wk01, xt1.., then the late consumers ----
        def load_w(dram, cols, tag):
            t = persist.tile([128, DCH * cols], BF16, tag=tag)
            a = dram[:, :]
            src = bass.AP(tensor=a.tensor, offset=a.offset,
                          ap=[[cols, 128], [128 * cols, DCH], [1, cols]])
            nc.sync.dma_start(t[:].rearrange("p (c f) -> p c f", c=DCH), src)
            return t

        wq01 = load_w(wq01_d, 128, "wq01")
        xt = []
        for d in range(DCH):
            t = persist.tile([128, S], BF16, tag=f"xt{d}")
            xt.append(t)
        # first chunks in 512-col pieces so the first projection matmuls can
        # start as soon as ~130KB (not 512KB) has landed
        for n in range(4):
            nsl = slice(n * 512, (n + 1) * 512)
            nc.sync.dma_start(xt[0][:, nsl], xT_d[0:128, nsl])
        wk01 = load_w(wk01_d, 128, "wk01")
        for n in range(4):
            nsl = slice(n * 512, (n + 1) * 512)
            nc.sync.dma_start(xt[1][:, nsl], xT_d[128:256, nsl])
        # all chunks in 512-col pieces: arrival granularity (~0.36us) then
        # matches the per-matmul consumption of the d-outer Q chain, so no
        # PE wait grows long enough to trip a HAM re-throttle mid-projection
        for d in range(2, DCH):
            for n in range(4):
                nsl = slice(n * 512, (n + 1) * 512)
                nc.sync.dma_start(xt[d][:, nsl],
                                  xT_d[d * 128:(d + 1) * 128, nsl])
        wqk2 = load_w(wqk2_d, 128, "wqk2")
        wv = load_w(wv_d, 195, "wv")

        def load_small(dram, shape, dt, tag):
            t = persist.tile(shape, dt, tag=tag)
            nc.sync.dma_start(t[:], dram[:, :])
            return t

        bq01 = load_small(bq01_d, [128, 1], F32, "bq01")
        bk01 = load_small(bk01_d, [128, 1], F32, "bk01")
        bqk2 = load_small(bqk2_d, [128, 1], F32, "bqk2")
        bv = load_small(bv_d, [128, 195], F32, "bv")
        tri = load_small(tri_d, [128, 128], BF16, "tri")

        wo01 = persist.tile([128, D], BF16, tag="wo01")
        nc.sync.dma_start(wo01[:], wo01_d[:, :])
        wo2 = persist.tile([64, D], BF16, tag="wo2")
        nc.sync.dma_start(wo2[:], wo2_d[:, :])

        # ---- persistent intermediates ----
        QT01 = persist.tile([128, S], BF16, tag="QT01")
        KT01 = persist.tile([128, S], BF16, tag="KT01")
        QKT2 = persist.tile([128, S], BF16, tag="QKT2")
        # partition swap of QKT2: rows 0-63 = K2, rows 64-127 = Q2*s
        QK2s = persist.tile([128, S], BF16, tag="QK2s")
        # V augmented with a ones column per head ([V_h | 1] x 3, 195 cols per
        # s-tile) so the PV matmul's 65th output row is the softmax denominator
        v_sb = persist.tile([128, NKT * 195], BF16, tag="v_sb")
        zT01 = persist.tile([128, S], BF16, tag="zT01")
        zT2 = persist.tile([64, S], BF16, tag="zT2")

        # ---- QKV projections. PSUM bank layout keeps the V/QK2 chains OFF
        # banks 0-3: the attention score slots alias banks 0-3, so they must
        # only WAR-depend on the (early-released) Q/K chains — otherwise the
        # first scores serialize behind the whole V projection. ----
        with tc.tile_pool(name="proj_ps", bufs=4, space="PSUM") as proj_pool, \
             tc.tile_pool(name="qk2_ps", bufs=2, space="PSUM") as qk2_pool, \
             tc.tile_pool(name="v_ps", bufs=2, space="PSUM") as v_pool:
            NS = S // 512
            for w_s, bias_s, out_s, nm in ((wq01, bq01, QT01, "q"),
                                           (wk01, bk01, KT01, "k")):
                pss = [proj_pool.tile([128, 512], F32, tag="chain",
                                      name=f"{nm}{n}") for n in range(NS)]
                for d in range(DCH):
                    lhsT = w_s[:, d * 128:(d + 1) * 128]
                    for n in range(NS):
                        nc.tensor.matmul(pss[n][:], lhsT,
                                         xt[d][:, n * 512:(n + 1) * 512],
                                         start=(d == 0), stop=(d == DCH - 1))
                for n in range(NS):
                    nsl = slice(n * 512, (n + 1) * 512)
                    nc.vector.tensor_scalar_add(out_s[:, nsl], pss[n][:],
                                                bias_s[:])

            # QK2 chain through its own 2 banks (4-5)
            for n in range(NS):
                ps = qk2_pool.tile([128, 512], F32, tag="qk2", name=f"h{n}")
                for d in range(DCH):
                    nc.tensor.matmul(ps[:], wqk2[:, d * 128:(d + 1) * 128],
                                     xt[d][:, n * 512:(n + 1) * 512],
                                     start=(d == 0), stop=(d == DCH - 1))
                nsl = slice(n * 512, (n + 1) * 512)
                nc.vector.tensor_scalar_add(QKT2[:, nsl], ps[:], bqk2[:])

            # h2 partition swap (scores h2 packs two j-tiles per issue slot:
            # rows 0-63 use (K2, Q2) from (QK2s, QKT2), rows 64-127 from
            # (QKT2, QK2s))
            nc.sync.dma_start(QK2s[0:64, :], QKT2[64:128, :])
            nc.sync.dma_start(QK2s[64:128, :], QKT2[0:64, :])

            # V through banks 6-7
            for s_t in range(NKT):
                ps = v_pool.tile([128, 195], F32, tag="vps")
                for d in range(DCH):
                    nc.tensor.matmul(ps[:], xt[d][:, s_t * 128:(s_t + 1) * 128],
                                     wv[:, d * 195:(d + 1) * 195],
                                     start=(d == 0), stop=(d == DCH - 1))
                nc.vector.tensor_add(v_sb[:, s_t * 195:(s_t + 1) * 195], ps[:],
                                     bv[:])

        # ---- attention ----
        # PSUM budget (8 banks): sT (scores [128,1024]) 2 slots = 4 banks;
        # zts 3 x [65,512] = 3 banks; outproj 1 bank ([128,512] sub-tiles).
        with tc.tile_pool(name="sT_ps", bufs=2, space="PSUM") as sT_pool, \
             tc.tile_pool(name="zT_ps", bufs=3, space="PSUM") as zT_pool, \
             tc.tile_pool(name="op_ps", bufs=1, space="PSUM") as op_pool, \
             tc.tile_pool(name="pt_sb", bufs=10) as pt_pool, \
             tc.tile_pool(name="rb_sb", bufs=3) as rb_pool, \
             tc.tile_pool(name="zu_sb", bufs=4) as zu_pool, \
             tc.tile_pool(name="zs_sb", bufs=2) as zs_pool, \
             tc.tile_pool(name="out_sb", bufs=4) as out_pool, \
             tc.tile_pool(name="recip_dr", bufs=4, space="DRAM") as rdr_pool, \
             tc.tile_pool(name="recip_sb", bufs=6) as recip_pool:

            # per-qi zts tiles, per stream: zts[qi] = [zt_h0, zt_h1] for the
            # pair stream, zt2s[qi] for the h2 stream
            zts = {}
            zt2s = {}

            def exp_mask(qi, g, st, name):
                pt = pt_pool.tile([128, G * 512], BF16, tag="pt", name=name)
                r0 = g * G - 4 * qi
                s0 = r0 * 128 if r0 >= 0 else 0
                nc.scalar.activation(pt[:, s0:G * 512], st[:, s0:G * 512],
                                     AF.Exp)
                for jj in range(G):
                    r = (g * G + jj) - 4 * qi
                    off = jj * 512
                    if r >= 0:
                        if r > 0:
                            nc.vector.memset(pt[:, off:off + r * 128], 0.0)
                        dsl = slice(off + r * 128, off + (r + 1) * 128)
                        nc.vector.tensor_mul(pt[:, dsl], pt[:, dsl], tri[:])
                return pt

            def pv(qi, g, pt, hv):
                J = 4 * qi + 4
                zt = zts[qi][hv] if hv < 2 else zt2s[qi]
                for jj in range(G):
                    j = g * G + jj
                    nc.tensor.matmul(
                        zt[:],
                        v_sb[:, j * 195 + hv * 65:j * 195 + hv * 65 + 65],
                        pt[:, jj * 512:(jj + 1) * 512],
                        start=(j == 0), stop=(j == J - 1))

            # round list: pair rounds (h0+h1) run ahead of h2 rounds so the
            # ScalarE exp pipeline starts as soon as Q,K are projected; h2
            # and PV trail behind the QK2/V projections.
            rounds = []
            for qi in range(NQB):
                NG = (4 * qi + 4) // G
                rounds += [("pair", qi, g) for g in range(NG)]
                if qi >= 1:
                    NGp = (4 * (qi - 1) + 4) // G
                    rounds += [("h2", qi - 1, g) for g in range(NGp)]
            rounds += [("h2", NQB - 1, g) for g in range((4 * NQB) // G)]

            def evac_pair(qi):
                # evacuate z (bf16) + denominator rows to SBUF right after
                # the last PV: releases the PSUM zts tiles within a round,
                # so the next q-block's PV never waits on the 1/sums chain
                zu = zu_pool.tile([64, 2 * 512], BF16, tag="zu")
                s3 = recip_pool.tile([1, 2 * 512], F32, tag="s3")
                for h in range(2):
                    hsl = slice(h * 512, (h + 1) * 512)
                    nc.vector.tensor_copy(zu[:, hsl], zts[qi][h][0:64, :])
                    nc.vector.tensor_copy(s3[:, hsl], zts[qi][h][64:65, :])
                return zu, s3

            def reshape_sums(s3, n, tag, eng=None):
                # phase A of the 1/sums chain: denominator row -> DRAM ->
                # [128, n/128] partition reshape
                eng = eng or nc.sync
                dr1 = rdr_pool.tile([1, n], F32, tag="dr1" + tag)
                eng.dma_start(dr1[:], s3[:])
                rs = recip_pool.tile([128, n // 128], F32, tag="rs" + tag)
                eng.dma_start(
                    rs[:], dr1[:].rearrange("o (p f) -> (o p) f", p=128))
                return rs

            def bcast_recip(rs, n, tag, eng=None):
                # phase B (deferred so the DVE reciprocal never FIFO-waits
                # on phase A's DMAs): reciprocal -> DRAM -> partition
                # broadcast
                eng = eng or nc.sync
                rr = recip_pool.tile([128, n // 128], F32, tag="rr" + tag)
                nc.vector.reciprocal(rr[:], rs[:])
                dr2 = rdr_pool.tile([1, n], F32, tag="dr2" + tag)
                eng.dma_start(
                    dr2[:].rearrange("o (p f) -> (o p) f", p=128), rr[:])
                rb = rb_pool.tile([64, n], F32, tag="rb" + tag)
                eng.dma_start(rb[:], dr2[0:1, :].broadcast_to([64, n]))
                return rb

            def mul_pair(qi, zu, rb):
                # zT_h = zu_h * broadcast(1/sums_h); on GpSimd (SBUF-only
                # operands) and deferred far enough that no FIFO that gates
                # the PV/exp pipeline ever waits on the chain
                qsl = slice(qi * QB, qi * QB + QB)
                nc.gpsimd.tensor_mul(zT01[0:64, qsl], zu[:, 0:512],
                                     rb[:, 0:512])
                z1 = zs_pool.tile([64, 512], BF16, tag="z1")
                nc.gpsimd.tensor_mul(z1[:], zu[:, 512:1024], rb[:, 512:1024])
                # head 1 lives on partitions 64-127 of zT01: DMA partition-shift
                nc.sync.dma_start(zT01[64:128, qsl], z1[:])

            def evac_h2(qi):
                zu = zu_pool.tile([64, 512], BF16, tag="zu2")
                s3 = recip_pool.tile([1, 512], F32, tag="s1")
                nc.vector.tensor_copy(zu[:], zt2s[qi][0:64, :])
                nc.vector.tensor_copy(s3[:], zt2s[qi][64:65, :])
                return zu, s3

            def mul_h2(qi, zu, rb):
                qsl = slice(qi * QB, qi * QB + QB)
                nc.gpsimd.tensor_mul(zT2[:, qsl], zu[:], rb[:])

            obs = {}

            def outproj_sub(t, n0, pool=None, tag="op"):
                # one 256-wide slice of q-tile t's output projection.
                # q0-q2: both heads' matmuls accumulate in one PSUM tile,
                # single CAST out. q3 (the tail): h2 projects UNNORMALIZED
                # z into the second half-bank; DVE folds 1/sums_h2 in as a
                # per-partition scalar so the tail never waits a broadcast.
                pool = pool or op_pool
                qi, c = t // 4, t % 4
                tsl = slice(t * 128, (t + 1) * 128)
                if t not in obs:
                    obs[t] = out_pool.tile([128, D], BF16, tag="ob",
                                           name=f"ob{t}")
                ob = obs[t]
                ps = pool.tile([128, 256], F32, tag=tag, name=f"op{t}_{n0}")
                nc.tensor.matmul(ps[:], zT01[:, tsl], wo01[:, n0:n0 + 256],
                                 start=True, stop=False)
                nc.tensor.matmul(ps[:], zT2[:, tsl], wo2[:, n0:n0 + 256],
                                 start=False, stop=True)
                nc.vector.tensor_copy(ob[:, n0:n0 + 256], ps[:])
                if n0 + 256 == D:
                    # output DMA via the idle GpSimd queue: keeps the Sync
                    # FIFO free for the latency-sensitive 1/sums chain hops
                    nc.gpsimd.dma_start(out_d[tsl, :], ob[:])

            # software pipeline with one round of skew: round r's PV is
            # emitted after round r+1's score matmuls, so the PE always
            # has independent work while ACT runs exp. acts[] holds
            # (delay_in_rounds, fn) items: normalize muls and outproj halves
            # are deferred enough rounds that their inputs (DMA reciprocal
            # chains) are ready when they reach an engine FIFO — a premature
            # instruction in a strict-FIFO queue stalls everything behind it.
            pending = None
            acts = []
            opq = []
            hp = ctx.enter_context(tc.high_priority())

            def flush_acts():
                nonlocal acts
                rest = []
                for dly, fn in acts:
                    if dly <= 1:
                        fn()
                    else:
                        rest.append((dly - 1, fn))
                acts = rest
                n_drain = 2 if len(opq) > 6 else 1
                for _ in range(min(n_drain, len(opq))):
                    t, n0 = opq.pop(0)
                    outproj_sub(t, n0)

            for kind, qi, g in rounds:
                q0 = qi * QB
                qsl = slice(q0, q0 + QB)
                if kind == "pair":
                    if g == 0:
                        zts[qi] = [zT_pool.tile([65, 512], F32, tag="zT",
                                                name=f"zt{qi}h{h}")
                                   for h in range(2)]
                    st0 = sT_pool.tile([128, G * 512], F32, tag="sT",
                                       name="st0")
                    st1 = sT_pool.tile([128, G * 512], F32, tag="sT",
                                       name="st1")
                    # adjacent emission, alternating row groups -> the PE
                    # overlaps each (st0, st1) pair via row packing
                    for jj in range(G):
                        j = g * G + jj
                        ksl = slice(j * 128, (j + 1) * 128)
                        osl = slice(jj * 512, (jj + 1) * 512)
                        nc.tensor.matmul(st0[:, osl], KT01[0:64, ksl],
                                         QT01[0:64, qsl],
                                         start=True, stop=True)
                        nc.tensor.matmul(st1[:, osl], KT01[64:128, ksl],
                                         QT01[64:128, qsl],
                                         start=True, stop=True)
                    sts = [(st0, 0), (st1, 1)]
                else:
                    if g == 0:
                        zt2s[qi] = zT_pool.tile([65, 512], F32, tag="zT",
                                                name=f"zt{qi}h2")
                    st2 = sT_pool.tile([128, G * 512], F32, tag="sT",
                                       name="st2")
                    # jj0 on rows 0-63 (K2 in QK2s, Q2 in QKT2), jj1 on rows
                    # 64-127 (K2 in QKT2, Q2 in QK2s): packed pair
                    j0 = g * G
                    nc.tensor.matmul(st2[:, 0:512],
                                     QK2s[0:64, j0 * 128:(j0 + 1) * 128],
                                     QKT2[0:64, qsl], start=True, stop=True)
                    nc.tensor.matmul(st2[:, 512:1024],
                                     QKT2[64:128, (j0 + 1) * 128:(j0 + 2) * 128],
                                     QK2s[64:128, qsl], start=True, stop=True)
                    sts = [(st2, 2)]

                if pending is not None:
                    pq, pg, plist = pending
                    for ppt, phv in plist:
                        pv(pq, pg, ppt, phv)
                    flush_acts()
                pending = (qi, g, [(exp_mask(qi, g, st, f"pt{hv}"), hv)
                                   for st, hv in sts])

                J = 4 * qi + 4
                if kind == "pair" and g == J // G - 1:
                    def pair_done(q=qi):
                        zu, s3 = evac_pair(q)
                        rs = reshape_sums(s3, 1024, "p")

                        def phase_b():
                            rb = bcast_recip(rs, 1024, "p")
                            acts.append((3, lambda: mul_pair(q, zu, rb)))
                        acts.append((3, phase_b))
                    acts.append((1, pair_done))
                if kind == "h2" and g == J // G - 1:
                    def h2_done(q=qi):
                        # final q-block's chain hops go via the (by then
                        # idle) ACT hwdge queue: no sync-FIFO contention on
                        # the tail's critical path
                        eng = nc.scalar if q == NQB - 1 else nc.sync
                        zu, s3 = evac_h2(q)
                        rs = reshape_sums(s3, 512, "h", eng=eng)

                        def phase_b():
                            rb = bcast_recip(rs, 512, "h", eng=eng)
                            acts.append((3, lambda: mul_h2(q, zu, rb)))
                        acts.append((3, phase_b))
                        acts.append((9, lambda: opq.extend(
                            (t, n0) for t in range(4 * q, 4 * q + 4)
                            for n0 in (0, 256, 512))))
                    acts.append((1, h2_done))

            pq, pg, plist = pending
            for ppt, phv in plist:
                pv(pq, pg, ppt, phv)
            # drain remaining deferred work; alternate the leftover outproj
            # halves across the freed sT slots and the op bank for a short
            # tail
            i = 0
            while acts or opq:
                rest = []
                for dly, fn in acts:
                    if dly <= 1:
                        fn()
                    else:
                        rest.append((dly - 1, fn))
                acts = rest
                if opq:
                    t, n0 = opq.pop(0)
                    pool, tag = ((sT_pool, "sT") if i % 3 else (op_pool, "op"))
                    outproj_sub(t, n0, pool=pool, tag=tag)
                    i += 1

    nc.compile()
    return nc


def _get_nc():
    global _BUILT
    if _BUILT is None:
        _BUILT = build_nc()
    return _BUILT


def make_in_maps(inputs):
    x = np.asarray(inputs["normalized_resid_pre"], dtype=np.float32)
    W_Q = np.asarray(inputs["W_Q"], dtype=np.float32)
    W_K = np.asarray(inputs["W_K"], dtype=np.float32)
    W_V = np.asarray(inputs["W_V"], dtype=np.float32)
    W_O = np.asarray(inputs["W_O"], dtype=np.float32)
    b_Q = np.asarray(inputs["b_Q"], dtype=np.float32)
    b_K = np.asarray(inputs["b_K"], dtype=np.float32)
    b_V = np.asarray(inputs["b_V"], dtype=np.float32)
    sc = 1.0 / np.sqrt(np.float32(DH))

    in_maps = []
    for c in range(N_CORES):
        b = c // 4
        h = (c % 4) * 3
        hs = [h, h + 1, h + 2]
        m = {
            "xT": np.ascontiguousarray(x[b].T).astype(NPBF16),
            "wq01": np.concatenate([W_Q[hs[0]] * sc, W_Q[hs[1]] * sc],
                                   axis=1).astype(NPBF16),
            "wk01": np.concatenate([W_K[hs[0]], W_K[hs[1]]], axis=1).astype(NPBF16),
            "wqk2": np.concatenate([W_Q[hs[2]] * sc, W_K[hs[2]]],
                                   axis=1).astype(NPBF16),
            "wv": np.concatenate(
                sum(([W_V[hh], np.zeros((D, 1), np.float32)] for hh in hs), []),
                axis=1).astype(NPBF16),
            "wo01": np.concatenate([W_O[hs[0]], W_O[hs[1]]], axis=0).astype(NPBF16),
            "wo2": W_O[hs[2]].astype(NPBF16),
            "bq01": (np.concatenate([b_Q[hs[0]], b_Q[hs[1]]]) * sc)[:, None]
                    .astype(np.float32),
            "bk01": np.concatenate([b_K[hs[0]], b_K[hs[1]]])[:, None]
                    .astype(np.float32),
            "bqk2": np.concatenate([b_Q[hs[2]] * sc, b_K[hs[2]]])[:, None]
                    .astype(np.float32),
            "bv": np.ascontiguousarray(np.broadcast_to(
                np.concatenate(
                    sum(([b_V[hh], np.ones(1, np.float32)] for hh in hs), [])),
                (128, 195))).astype(np.float32),
        }
        in_maps.append(m)
    return in_maps


def kernel(**inputs):
    global LAST_EXEC_NS, LAST_RESULT
    nc = _get_nc()
    in_maps = make_in_maps(inputs)
    b_O = np.asarray(inputs["b_O"], dtype=np.float32)

    res = run_bass_kernel_spmd(nc, in_maps, core_ids=list(range(N_CORES)),
                               trace=TRACE_ENABLED)
    LAST_EXEC_NS = res.exec_time_ns
    LAST_RESULT = res
    parts = [np.asarray(r["out_p"], dtype=np.float32) for r in res.results]
    out0 = parts[0] + parts[1] + parts[2] + parts[3]
    out1 = parts[4] + parts[5] + parts[6] + parts[7]
    out = np.stack([out0, out1]) + b_O
    return out.astype(np.float32)


# revision 74
# speedup vs baseline: 1.2049x; 1.2049x over previous
"""Multi-head causal attention (dense transformer block) on 8 TRN2 NeuronCores.

Problem: B=2, S=2048, D_MODEL=768, H=12 heads, D_HEAD=64, fp32 I/O.

Sharding: 24 (batch, head) units over 8 cores -> 3 heads x 1 batch per core.
Cores 0-3 handle batch 0 (heads 0-2, 3-5, 6-8, 9-11), cores 4-7 batch 1.
Each core computes its heads' contribution to out[b] = sum_h z_h @ W_O[h];
the host sums the 4 bf16 partials per batch in f32 and adds b_O.

Per-core dataflow (all matmuls bf16 -> fp32 PSUM):
  - x[b]^T staged in SBUF as 6 chunks [128, 2048].
  - QT/KT in "head-transposed" layout [64, S] packed 2-per-tile:
      QT01 = [Q_h0*s | Q_h1*s]^T x  (scale 1/sqrt(64) folded into W_Q/b_Q)
      KT01 = [K_h0 | K_h1]^T x, QKT2 = [Q_h2*s | K_h2]^T x
    plus QK2swap = partition-swap of QKT2 so h2 has Q and K on BOTH
    partition halves.
  - scores TRANSPOSED: sT[k, q] = KT^T-slice x QT (K=64 contraction).
    The two K=64 matmuls of a (h0, h1) pair are emitted adjacently with
    disjoint row-groups (partitions 0-63 / 64-127) so the PE runs them
    CONCURRENTLY via tile_position row packing; h2 packs its two j-tiles
    the same way using QK2swap.
  - h0/h1 rounds and h2 rounds are separate streams: pair rounds only need
    Q,K (projected first), so ScalarE exp work starts early; h2 rounds and
    PV lag until QK2/V land without stalling ACT.
  - exp via ScalarE (one instr per [128, 1024] j-pair), causal mask via
    memset + one triangular-mask multiply per diagonal 128x128 tile.
  - zT[h] = sum_j [V_h[j] | 1]^T x PT[j]  (unnormalized; the ones column
    makes output row 64 the softmax denominator), normalized as
    zT * broadcast(1/sums) when copying PSUM->SBUF.
  - output projection per q-block as soon as its zT is normalized:
    out[q, :] = zT01^T-slice @ WO01 + zT2^T-slice @ WO2, copied to SBUF as
    bf16 and DMA'd out immediately (halves the output-DMA tail).
"""

import numpy as np
import ml_dtypes
from contextlib import ExitStack

import concourse.bass as bass
import concourse.mybir as mybir
import concourse.tile as tile
from concourse import bacc
from concourse.bass_utils import run_bass_kernel_spmd

BF16 = mybir.dt.bfloat16
F32 = mybir.dt.float32
AF = mybir.ActivationFunctionType
NPBF16 = ml_dtypes.bfloat16

B, S, D, H, DH = 2, 2048, 768, 12, 64
N_CORES = 8
DCH = D // 128          # 6 d_model chunks
NKT = S // 128          # 16 k tiles
QB = 512                # q block width
NQB = S // QB           # 4 q blocks
G = 2                   # k-tiles per exp group

TRACE_ENABLED = False
LAST_EXEC_NS = None
LAST_RESULT = None
_BUILT = None


def build_nc():
    nc = bacc.Bacc("TRN2", target_bir_lowering=False, debug=False)

    xT_d = nc.dram_tensor("xT", [D, S], BF16, kind="ExternalInput")
    wq01_d = nc.dram_tensor("wq01", [D, 128], BF16, kind="ExternalInput")
    wk01_d = nc.dram_tensor("wk01", [D, 128], BF16, kind="ExternalInput")
    wqk2_d = nc.dram_tensor("wqk2", [D, 128], BF16, kind="ExternalInput")
    wv_d = nc.dram_tensor("wv", [D, 195], BF16, kind="ExternalInput")
    wo01_d = nc.dram_tensor("wo01", [128, D], BF16, kind="ExternalInput")
    wo2_d = nc.dram_tensor("wo2", [64, D], BF16, kind="ExternalInput")
    bq01_d = nc.dram_tensor("bq01", [128, 1], F32, kind="ExternalInput")
    bk01_d = nc.dram_tensor("bk01", [128, 1], F32, kind="ExternalInput")
    bqk2_d = nc.dram_tensor("bqk2", [128, 1], F32, kind="ExternalInput")
    bv_d = nc.dram_tensor("bv", [128, 195], F32, kind="ExternalInput")
    out_d = nc.dram_tensor("out_p", [S, D], BF16, kind="ExternalOutput")

    tri_np = np.triu(np.ones((128, 128), np.float32)).astype(NPBF16)
    tri_d = nc.inline_tensor(tri_np, "tri")

    with tile.TileContext(nc) as tc, ExitStack() as ctx:
        persist = ctx.enter_context(tc.tile_pool(name="persist", bufs=1))

        # ---- HAM warm-up: ~5us of dummy matmuls spanning the input-DMA
        # wait, so the PE clock is at 2.4GHz when the first projection
        # matmuls issue ----
        with tc.tile_pool(name="warm_ps", bufs=1, space="PSUM") as warm_pool:
            wz = persist.tile([128, 128], BF16, tag="wz")
            nc.vector.memset(wz[:], 0.0)
            wps = warm_pool.tile([128, 128], F32, tag="warm")
            for _ in range(40):
                nc.tensor.matmul(wps[:], wz[:], wz[:], start=True, stop=True)

        # ---- stage inputs in SBUF. DMA issue order feeds the d-outer
        # Q+K projection: wq01, xt0, wk01, xt1.., then the late consumers ----
        def load_w(dram, cols, tag):
            t = persist.tile([128, DCH * cols], BF16, tag=tag)
            a = dram[:, :]
            src = bass.AP(tensor=a.tensor, offset=a.offset,
                          ap=[[cols, 128], [128 * cols, DCH], [1, cols]])
            nc.sync.dma_start(t[:].rearrange("p (c f) -> p c f", c=DCH), src)
            return t

        wq01 = load_w(wq01_d, 128, "wq01")
        xt = []
        for d in range(DCH):
            t = persist.tile([128, S], BF16, tag=f"xt{d}")
            xt.append(t)
        # first chunks in 512-col pieces so the first projection matmuls can
        # start as soon as ~130KB (not 512KB) has landed
        for n in range(4):
            nsl = slice(n * 512, (n + 1) * 512)
            nc.sync.dma_start(xt[0][:, nsl], xT_d[0:128, nsl])
        wk01 = load_w(wk01_d, 128, "wk01")
        for n in range(4):
            nsl = slice(n * 512, (n + 1) * 512)
            nc.sync.dma_start(xt[1][:, nsl], xT_d[128:256, nsl])
        for d in range(2, DCH):
            nc.sync.dma_start(xt[d][:], xT_d[d * 128:(d + 1) * 128, :])
        wqk2 = load_w(wqk2_d, 128, "wqk2")
        wv = load_w(wv_d, 195, "wv")

        def load_small(dram, shape, dt, tag):
            t = persist.tile(shape, dt, tag=tag)
            nc.sync.dma_start(t[:], dram[:, :])
            return t

        bq01 = load_small(bq01_d, [128, 1], F32, "bq01")
        bk01 = load_small(bk01_d, [128, 1], F32, "bk01")
        bqk2 = load_small(bqk2_d, [128, 1], F32, "bqk2")
        bv = load_small(bv_d, [128, 195], F32, "bv")
        tri = load_small(tri_d, [128, 128], BF16, "tri")

        wo01 = persist.tile([128, D], BF16, tag="wo01")
        nc.sync.dma_start(wo01[:], wo01_d[:, :])
        wo2 = persist.tile([64, D], BF16, tag="wo2")
        nc.sync.dma_start(wo2[:], wo2_d[:, :])

        # ---- persistent intermediates ----
        QT01 = persist.tile([128, S], BF16, tag="QT01")
        KT01 = persist.tile([128, S], BF16, tag="KT01")
        QKT2 = persist.tile([128, S], BF16, tag="QKT2")
        # partition swap of QKT2: rows 0-63 = K2, rows 64-127 = Q2*s
        QK2s = persist.tile([128, S], BF16, tag="QK2s")
        # V augmented with a ones column per head ([V_h | 1] x 3, 195 cols per
        # s-tile) so the PV matmul's 65th output row is the softmax denominator
        v_sb = persist.tile([128, NKT * 195], BF16, tag="v_sb")
        zT01 = persist.tile([128, S], BF16, tag="zT01")
        zT2 = persist.tile([64, S], BF16, tag="zT2")

        # ---- QKV projections. PSUM bank layout keeps the V/QK2 chains OFF
        # banks 0-3: the attention score slots alias banks 0-3, so they must
        # only WAR-depend on the (early-released) Q/K chains — otherwise the
        # first scores serialize behind the whole V projection. ----
        with tc.tile_pool(name="proj_ps", bufs=4, space="PSUM") as proj_pool, \
             tc.tile_pool(name="qk2_ps", bufs=2, space="PSUM") as qk2_pool, \
             tc.tile_pool(name="v_ps", bufs=2, space="PSUM") as v_pool:
            NS = S // 512
            for w_s, bias_s, out_s, nm in ((wq01, bq01, QT01, "q"),
                                           (wk01, bk01, KT01, "k")):
                pss = [proj_pool.tile([128, 512], F32, tag="chain",
                                      name=f"{nm}{n}") for n in range(NS)]
                for d in range(DCH):
                    lhsT = w_s[:, d * 128:(d + 1) * 128]
                    for n in range(NS):
                        nc.tensor.matmul(pss[n][:], lhsT,
                                         xt[d][:, n * 512:(n + 1) * 512],
                                         start=(d == 0), stop=(d == DCH - 1))
                for n in range(NS):
                    nsl = slice(n * 512, (n + 1) * 512)
                    nc.vector.tensor_scalar_add(out_s[:, nsl], pss[n][:],
                                                bias_s[:])

            # QK2 chain through its own 2 banks (4-5)
            for n in range(NS):
                ps = qk2_pool.tile([128, 512], F32, tag="qk2", name=f"h{n}")
                for d in range(DCH):
                    nc.tensor.matmul(ps[:], wqk2[:, d * 128:(d + 1) * 128],
                                     xt[d][:, n * 512:(n + 1) * 512],
                                     start=(d == 0), stop=(d == DCH - 1))
                nsl = slice(n * 512, (n + 1) * 512)
                nc.vector.tensor_scalar_add(QKT2[:, nsl], ps[:], bqk2[:])

            # h2 partition swap (scores h2 packs two j-tiles per issue slot:
            # rows 0-63 use (K2, Q2) from (QK2s, QKT2), rows 64-127 from
            # (QKT2, QK2s))
            nc.sync.dma_start(QK2s[0:64, :], QKT2[64:128, :])
            nc.sync.dma_start(QK2s[64:128, :], QKT2[0:64, :])

            # V through banks 6-7
            for s_t in range(NKT):
                ps = v_pool.tile([128, 195], F32, tag="vps")
                for d in range(DCH):
                    nc.tensor.matmul(ps[:], xt[d][:, s_t * 128:(s_t + 1) * 128],
                                     wv[:, d * 195:(d + 1) * 195],
                                     start=(d == 0), stop=(d == DCH - 1))
                nc.vector.tensor_add(v_sb[:, s_t * 195:(s_t + 1) * 195], ps[:],
                                     bv[:])

        # ---- attention ----
        # PSUM budget (8 banks): sT (scores [128,1024]) 2 slots = 4 banks;
        # zts 3 x [65,512] = 3 banks; outproj 1 bank ([128,512] sub-tiles).
        with tc.tile_pool(name="sT_ps", bufs=2, space="PSUM") as sT_pool, \
             tc.tile_pool(name="zT_ps", bufs=3, space="PSUM") as zT_pool, \
             tc.tile_pool(name="op_ps", bufs=1, space="PSUM") as op_pool, \
             tc.tile_pool(name="pt_sb", bufs=10) as pt_pool, \
             tc.tile_pool(name="rb_sb", bufs=3) as rb_pool, \
             tc.tile_pool(name="zu_sb", bufs=4) as zu_pool, \
             tc.tile_pool(name="zs_sb", bufs=2) as zs_pool, \
             tc.tile_pool(name="out_sb", bufs=4) as out_pool, \
             tc.tile_pool(name="recip_dr", bufs=4, space="DRAM") as rdr_pool, \
             tc.tile_pool(name="recip_sb", bufs=6) as recip_pool:

            # per-qi zts tiles, per stream: zts[qi] = [zt_h0, zt_h1] for the
            # pair stream, zt2s[qi] for the h2 stream
            zts = {}
            zt2s = {}

            def exp_mask(qi, g, st, name):
                pt = pt_pool.tile([128, G * 512], BF16, tag="pt", name=name)
                r0 = g * G - 4 * qi
                s0 = r0 * 128 if r0 >= 0 else 0
                nc.scalar.activation(pt[:, s0:G * 512], st[:, s0:G * 512],
                                     AF.Exp)
                for jj in range(G):
                    r = (g * G + jj) - 4 * qi
                    off = jj * 512
                    if r >= 0:
                        if r > 0:
                            nc.vector.memset(pt[:, off:off + r * 128], 0.0)
                        dsl = slice(off + r * 128, off + (r + 1) * 128)
                        nc.vector.tensor_mul(pt[:, dsl], pt[:, dsl], tri[:])
                return pt

            def pv(qi, g, pt, hv):
                J = 4 * qi + 4
                zt = zts[qi][hv] if hv < 2 else zt2s[qi]
                for jj in range(G):
                    j = g * G + jj
                    nc.tensor.matmul(
                        zt[:],
                        v_sb[:, j * 195 + hv * 65:j * 195 + hv * 65 + 65],
                        pt[:, jj * 512:(jj + 1) * 512],
                        start=(j == 0), stop=(j == J - 1))

            # round list: pair rounds (h0+h1) run ahead of h2 rounds so the
            # ScalarE exp pipeline starts as soon as Q,K are projected; h2
            # and PV trail behind the QK2/V projections.
            rounds = []
            for qi in range(NQB):
                NG = (4 * qi + 4) // G
                rounds += [("pair", qi, g) for g in range(NG)]
                if qi >= 1:
                    NGp = (4 * (qi - 1) + 4) // G
                    rounds += [("h2", qi - 1, g) for g in range(NGp)]
            rounds += [("h2", NQB - 1, g) for g in range((4 * NQB) // G)]

            def evac_pair(qi):
                # evacuate z (bf16) + denominator rows to SBUF right after
                # the last PV: releases the PSUM zts tiles within a round,
                # so the next q-block's PV never waits on the 1/sums chain
                zu = zu_pool.tile([64, 2 * 512], BF16, tag="zu")
                s3 = recip_pool.tile([1, 2 * 512], F32, tag="s3")
                for h in range(2):
                    hsl = slice(h * 512, (h + 1) * 512)
                    nc.vector.tensor_copy(zu[:, hsl], zts[qi][h][0:64, :])
                    nc.vector.tensor_copy(s3[:, hsl], zts[qi][h][64:65, :])
                return zu, s3

            def reshape_sums(s3, n, tag, eng=None):
                # phase A of the 1/sums chain: denominator row -> DRAM ->
                # [128, n/128] partition reshape
                eng = eng or nc.sync
                dr1 = rdr_pool.tile([1, n], F32, tag="dr1" + tag)
                eng.dma_start(dr1[:], s3[:])
                rs = recip_pool.tile([128, n // 128], F32, tag="rs" + tag)
                eng.dma_start(
                    rs[:], dr1[:].rearrange("o (p f) -> (o p) f", p=128))
                return rs

            def bcast_recip(rs, n, tag, eng=None):
                # phase B (deferred so the DVE reciprocal never FIFO-waits
                # on phase A's DMAs): reciprocal -> DRAM -> partition
                # broadcast
                eng = eng or nc.sync
                rr = recip_pool.tile([128, n // 128], F32, tag="rr" + tag)
                nc.vector.reciprocal(rr[:], rs[:])
                dr2 = rdr_pool.tile([1, n], F32, tag="dr2" + tag)
                eng.dma_start(
                    dr2[:].rearrange("o (p f) -> (o p) f", p=128), rr[:])
                rb = rb_pool.tile([64, n], F32, tag="rb" + tag)
                eng.dma_start(rb[:], dr2[0:1, :].broadcast_to([64, n]))
                return rb

            def mul_pair(qi, zu, rb):
                # zT_h = zu_h * broadcast(1/sums_h); on GpSimd (SBUF-only
                # operands) and deferred far enough that no FIFO that gates
                # the PV/exp pipeline ever waits on the chain
                qsl = slice(qi * QB, qi * QB + QB)
                nc.gpsimd.tensor_mul(zT01[0:64, qsl], zu[:, 0:512],
                                     rb[:, 0:512])
                z1 = zs_pool.tile([64, 512], BF16, tag="z1")
                nc.gpsimd.tensor_mul(z1[:], zu[:, 512:1024], rb[:, 512:1024])
                # head 1 lives on partitions 64-127 of zT01: DMA partition-shift
                nc.sync.dma_start(zT01[64:128, qsl], z1[:])

            def evac_h2(qi):
                zu = zu_pool.tile([64, 512], BF16, tag="zu2")
                s3 = recip_pool.tile([1, 512], F32, tag="s1")
                nc.vector.tensor_copy(zu[:], zt2s[qi][0:64, :])
                nc.vector.tensor_copy(s3[:], zt2s[qi][64:65, :])
                return zu, s3

            def mul_h2(qi, zu, rb):
                qsl = slice(qi * QB, qi * QB + QB)
                nc.gpsimd.tensor_mul(zT2[:, qsl], zu[:], rb[:])

            obs = {}

            def outproj_sub(t, n0, pool=None, tag="op"):
                # one 256-wide slice of q-tile t's output projection.
                # q0-q2: both heads' matmuls accumulate in one PSUM tile,
                # single CAST out. q3 (the tail): h2 projects UNNORMALIZED
                # z into the second half-bank; DVE folds 1/sums_h2 in as a
                # per-partition scalar so the tail never waits a broadcast.
                pool = pool or op_pool
                qi, c = t // 4, t % 4
                tsl = slice(t * 128, (t + 1) * 128)
                if t not in obs:
                    obs[t] = out_pool.tile([128, D], BF16, tag="ob",
                                           name=f"ob{t}")
                ob = obs[t]
                ps = pool.tile([128, 256], F32, tag=tag, name=f"op{t}_{n0}")
                nc.tensor.matmul(ps[:], zT01[:, tsl], wo01[:, n0:n0 + 256],
                                 start=True, stop=False)
                nc.tensor.matmul(ps[:], zT2[:, tsl], wo2[:, n0:n0 + 256],
                                 start=False, stop=True)
                nc.vector.tensor_copy(ob[:, n0:n0 + 256], ps[:])
                if n0 + 256 == D:
                    # output DMA via the idle GpSimd queue: keeps the Sync
                    # FIFO free for the latency-sensitive 1/sums chain hops
                    nc.gpsimd.dma_start(out_d[tsl, :], ob[:])

            # software pipeline with one round of skew: round r's PV is
            # emitted after round r+1's score matmuls, so the PE always
            # has independent work while ACT runs exp. acts[] holds
            # (delay_in_rounds, fn) items: normalize muls and outproj halves
            # are deferred enough rounds that their inputs (DMA reciprocal
            # chains) are ready when they reach an engine FIFO — a premature
            # instruction in a strict-FIFO queue stalls everything behind it.
            pending = None
            acts = []
            opq = []
            hp = ctx.enter_context(tc.high_priority())

            def flush_acts():
                nonlocal acts
                rest = []
                for dly, fn in acts:
                    if dly <= 1:
                        fn()
                    else:
                        rest.append((dly - 1, fn))
                acts = rest
                n_drain = 2 if len(opq) > 6 else 1
                for _ in range(min(n_drain, len(opq))):
                    t, n0 = opq.pop(0)
                    outproj_sub(t, n0)

            for kind, qi, g in rounds:
                q0 = qi * QB
                qsl = slice(q0, q0 + QB)
                if kind == "pair":
                    if g == 0:
                        zts[qi] = [zT_pool.tile([65, 512], F32, tag="zT",
                                                name=f"zt{qi}h{h}")
                                   for h in range(2)]
                    st0 = sT_pool.tile([128, G * 512], F32, tag="sT",
                                       name="st0")
                    st1 = sT_pool.tile([128, G * 512], F32, tag="sT",
                                       name="st1")
                    # adjacent emission, alternating row groups -> the PE
                    # overlaps each (st0, st1) pair via row packing
                    for jj in range(G):
                        j = g * G + jj
                        ksl = slice(j * 128, (j + 1) * 128)
                        osl = slice(jj * 512, (jj + 1) * 512)
                        nc.tensor.matmul(st0[:, osl], KT01[0:64, ksl],
                                         QT01[0:64, qsl],
                                         start=True, stop=True)
                        nc.tensor.matmul(st1[:, osl], KT01[64:128, ksl],
                                         QT01[64:128, qsl],
                                         start=True, stop=True)
                    sts = [(st0, 0), (st1, 1)]
                else:
                    if g == 0:
                        zt2s[qi] = zT_pool.tile([65, 512], F32, tag="zT",
                                                name=f"zt{qi}h2")
                    st2 = sT_pool.tile([128, G * 512], F32, tag="sT",
                                       name="st2")
                    # jj0 on rows 0-63 (K2 in QK2s, Q2 in QKT2), jj1 on rows
                    # 64-127 (K2 in QKT2, Q2 in QK2s): packed pair
                    j0 = g * G
                    nc.tensor.matmul(st2[:, 0:512],
                                     QK2s[0:64, j0 * 128:(j0 + 1) * 128],
                                     QKT2[0:64, qsl], start=True, stop=True)
                    nc.tensor.matmul(st2[:, 512:1024],
                                     QKT2[64:128, (j0 + 1) * 128:(j0 + 2) * 128],
                                     QK2s[64:128, qsl], start=True, stop=True)
                    sts = [(st2, 2)]

                if pending is not None:
                    pq, pg, plist = pending
                    for ppt, phv in plist:
                        pv(pq, pg, ppt, phv)
                    flush_acts()
                pending = (qi, g, [(exp_mask(qi, g, st, f"pt{hv}"), hv)
                                   for st, hv in sts])

                J = 4 * qi + 4
                if kind == "pair" and g == J // G - 1:
                    def pair_done(q=qi):
                        zu, s3 = evac_pair(q)
                        rs = reshape_sums(s3, 1024, "p")

                        def phase_b():
                            rb = bcast_recip(rs, 1024, "p")
                            acts.append((3, lambda: mul_pair(q, zu, rb)))
                        acts.append((3, phase_b))
                    acts.append((1, pair_done))
                if kind == "h2" and g == J // G - 1:
                    def h2_done(q=qi):
                        # final q-block's chain hops go via the (by then
                        # idle) ACT hwdge queue: no sync-FIFO contention on
                        # the tail's critical path
                        eng = nc.scalar if q == NQB - 1 else nc.sync
                        zu, s3 = evac_h2(q)
                        rs = reshape_sums(s3, 512, "h", eng=eng)

                        def phase_b():
                            rb = bcast_recip(rs, 512, "h", eng=eng)
                            acts.append((3, lambda: mul_h2(q, zu, rb)))
                        acts.append((3, phase_b))
                        acts.append((9, lambda: opq.extend(
                            (t, n0) for t in range(4 * q, 4 * q + 4)
                            for n0 in (0, 256, 512))))
                    acts.append((1, h2_done))

            pq, pg, plist = pending
            for ppt, phv in plist:
                pv(pq, pg, ppt, phv)
            # drain remaining deferred work; alternate the leftover outproj
            # halves across the freed sT slots and the op bank for a short
            # tail
            i = 0
            while acts or opq:
                rest = []
                for dly, fn in acts:
                    if dly <= 1:
                        fn()
                    else:
                        rest.append((dly - 1, fn))
                acts = rest
                if opq:
                    t, n0 = opq.pop(0)
                    pool, tag = ((sT_pool, "sT") if i % 3 else (op_pool, "op"))
                    outproj_sub(t, n0, pool=pool, tag=tag)
                    i += 1

    nc.compile()
    return nc


def _get_nc():
    global _BUILT
    if _BUILT is None:
        _BUILT = build_nc()
    return _BUILT


def make_in_maps(inputs):
    x = np.asarray(inputs["normalized_resid_pre"], dtype=np.float32)
    W_Q = np.asarray(inputs["W_Q"], dtype=np.float32)
    W_K = np.asarray(inputs["W_K"], dtype=np.float32)
    W_V = np.asarray(inputs["W_V"], dtype=np.float32)
    W_O = np.asarray(inputs["W_O"], dtype=np.float32)
    b_Q = np.asarray(inputs["b_Q"], dtype=np.float32)
    b_K = np.asarray(inputs["b_K"], dtype=np.float32)
    b_V = np.asarray(inputs["b_V"], dtype=np.float32)
    sc = 1.0 / np.sqrt(np.float32(DH))

    in_maps = []
    for c in range(N_CORES):
        b = c // 4
        h = (c % 4) * 3
        hs = [h, h + 1, h + 2]
        m = {
            "xT": np.ascontiguousarray(x[b].T).astype(NPBF16),
            "wq01": np.concatenate([W_Q[hs[0]] * sc, W_Q[hs[1]] * sc],
                                   axis=1).astype(NPBF16),
            "wk01": np.concatenate([W_K[hs[0]], W_K[hs[1]]], axis=1).astype(NPBF16),
            "wqk2": np.concatenate([W_Q[hs[2]] * sc, W_K[hs[2]]],
                                   axis=1).astype(NPBF16),
            "wv": np.concatenate(
                sum(([W_V[hh], np.zeros((D, 1), np.float32)] for hh in hs), []),
                axis=1).astype(NPBF16),
            "wo01": np.concatenate([W_O[hs[0]], W_O[hs[1]]], axis=0).astype(NPBF16),
            "wo2": W_O[hs[2]].astype(NPBF16),
            "bq01": (np.concatenate([b_Q[hs[0]], b_Q[hs[1]]]) * sc)[:, None]
                    .astype(np.float32),
            "bk01": np.concatenate([b_K[hs[0]], b_K[hs[1]]])[:, None]
                    .astype(np.float32),
            "bqk2": np.concatenate([b_Q[hs[2]] * sc, b_K[hs[2]]])[:, None]
                    .astype(np.float32),
            "bv": np.ascontiguousarray(np.broadcast_to(
                np.concatenate(
                    sum(([b_V[hh], np.ones(1, np.float32)] for hh in hs), [])),
                (128, 195))).astype(np.float32),
        }
        in_maps.append(m)
    return in_maps


def kernel(**inputs):
    global LAST_EXEC_NS, LAST_RESULT
    nc = _get_nc()
    in_maps = make_in_maps(inputs)
    b_O = np.asarray(inputs["b_O"], dtype=np.float32)

    res = run_bass_kernel_spmd(nc, in_maps, core_ids=list(range(N_CORES)),
                               trace=TRACE_ENABLED)
    LAST_EXEC_NS = res.exec_time_ns
    LAST_RESULT = res
    parts = [np.asarray(r["out_p"], dtype=np.float32) for r in res.results]
    out0 = parts[0] + parts[1] + parts[2] + parts[3]
    out1 = parts[4] + parts[5] + parts[6] + parts[7]
    out = np.stack([out0, out1]) + b_O
    return out.astype(np.float32)


# revision 77
# speedup vs baseline: 1.2208x; 1.0132x over previous
"""Multi-head causal attention (dense transformer block) on 8 TRN2 NeuronCores.

Problem: B=2, S=2048, D_MODEL=768, H=12 heads, D_HEAD=64, fp32 I/O.

Sharding: 24 (batch, head) units over 8 cores -> 3 heads x 1 batch per core.
Cores 0-3 handle batch 0 (heads 0-2, 3-5, 6-8, 9-11), cores 4-7 batch 1.
Each core computes its heads' contribution to out[b] = sum_h z_h @ W_O[h];
the host sums the 4 bf16 partials per batch in f32 and adds b_O.

Per-core dataflow (all matmuls bf16 -> fp32 PSUM):
  - x[b]^T staged in SBUF as 6 chunks [128, 2048].
  - QT/KT in "head-transposed" layout [64, S] packed 2-per-tile:
      QT01 = [Q_h0*s | Q_h1*s]^T x  (scale 1/sqrt(64) folded into W_Q/b_Q)
      KT01 = [K_h0 | K_h1]^T x, QKT2 = [Q_h2*s | K_h2]^T x
    plus QK2swap = partition-swap of QKT2 so h2 has Q and K on BOTH
    partition halves.
  - scores TRANSPOSED: sT[k, q] = KT^T-slice x QT (K=64 contraction).
    The two K=64 matmuls of a (h0, h1) pair are emitted adjacently with
    disjoint row-groups (partitions 0-63 / 64-127) so the PE runs them
    CONCURRENTLY via tile_position row packing; h2 packs its two j-tiles
    the same way using QK2swap.
  - h0/h1 rounds and h2 rounds are separate streams: pair rounds only need
    Q,K (projected first), so ScalarE exp work starts early; h2 rounds and
    PV lag until QK2/V land without stalling ACT.
  - exp via ScalarE (one instr per [128, 1024] j-pair), causal mask via
    memset + one triangular-mask multiply per diagonal 128x128 tile.
  - zT[h] = sum_j [V_h[j] | 1]^T x PT[j]  (unnormalized; the ones column
    makes output row 64 the softmax denominator), normalized as
    zT * broadcast(1/sums) when copying PSUM->SBUF.
  - output projection per q-block as soon as its zT is normalized:
    out[q, :] = zT01^T-slice @ WO01 + zT2^T-slice @ WO2, copied to SBUF as
    bf16 and DMA'd out immediately (halves the output-DMA tail).
"""

import numpy as np
import ml_dtypes
from contextlib import ExitStack

import concourse.bass as bass
import concourse.mybir as mybir
import concourse.tile as tile
from concourse import bacc
from concourse.bass_utils import run_bass_kernel_spmd

BF16 = mybir.dt.bfloat16
F32 = mybir.dt.float32
AF = mybir.ActivationFunctionType
NPBF16 = ml_dtypes.bfloat16

B, S, D, H, DH = 2, 2048, 768, 12, 64
N_CORES = 8
DCH = D // 128          # 6 d_model chunks
NKT = S // 128          # 16 k tiles
QB = 512                # q block width
NQB = S // QB           # 4 q blocks
G = 2                   # k-tiles per exp group

TRACE_ENABLED = False
LAST_EXEC_NS = None
LAST_RESULT = None
_BUILT = None


def build_nc():
    nc = bacc.Bacc("TRN2", target_bir_lowering=False, debug=False)

    xT_d = nc.dram_tensor("xT", [D, S], BF16, kind="ExternalInput")
    wq01_d = nc.dram_tensor("wq01", [D, 128], BF16, kind="ExternalInput")
    wk01_d = nc.dram_tensor("wk01", [D, 128], BF16, kind="ExternalInput")
    wqk2_d = nc.dram_tensor("wqk2", [D, 128], BF16, kind="ExternalInput")
    wv_d = nc.dram_tensor("wv", [D, 195], BF16, kind="ExternalInput")
    wo01_d = nc.dram_tensor("wo01", [128, D], BF16, kind="ExternalInput")
    wo2_d = nc.dram_tensor("wo2", [64, D], BF16, kind="ExternalInput")
    bq01_d = nc.dram_tensor("bq01", [128, 1], F32, kind="ExternalInput")
    bk01_d = nc.dram_tensor("bk01", [128, 1], F32, kind="ExternalInput")
    bqk2_d = nc.dram_tensor("bqk2", [128, 1], F32, kind="ExternalInput")
    bv_d = nc.dram_tensor("bv", [128, 195], F32, kind="ExternalInput")
    out_d = nc.dram_tensor("out_p", [S, D], BF16, kind="ExternalOutput")

    tri_np = np.triu(np.ones((128, 128), np.float32)).astype(NPBF16)
    tri_d = nc.inline_tensor(tri_np, "tri")

    with tile.TileContext(nc) as tc, ExitStack() as ctx:
        persist = ctx.enter_context(tc.tile_pool(name="persist", bufs=1))

        # ---- HAM warm-up: ~5us of dummy matmuls spanning the input-DMA
        # wait, so the PE clock is at 2.4GHz when the first projection
        # matmuls issue ----
        with tc.tile_pool(name="warm_ps", bufs=1, space="PSUM") as warm_pool:
            wz = persist.tile([128, 128], BF16, tag="wz")
            nc.vector.memset(wz[:], 0.0)
            wps = warm_pool.tile([128, 128], F32, tag="warm")
            for _ in range(40):
                nc.tensor.matmul(wps[:], wz[:], wz[:], start=True, stop=True)

        # ---- stage inputs in SBUF. DMA issue order feeds the d-outer
        # Q+K projection: wq01, xt0, wk01, xt1.., then the late consumers ----
        def load_w(dram, cols, tag):
            t = persist.tile([128, DCH * cols], BF16, tag=tag)
            a = dram[:, :]
            src = bass.AP(tensor=a.tensor, offset=a.offset,
                          ap=[[cols, 128], [128 * cols, DCH], [1, cols]])
            nc.sync.dma_start(t[:].rearrange("p (c f) -> p c f", c=DCH), src)
            return t

        wq01 = load_w(wq01_d, 128, "wq01")
        xt = []
        for d in range(DCH):
            t = persist.tile([128, S], BF16, tag=f"xt{d}")
            xt.append(t)
        # first chunks in 512-col pieces so the first projection matmuls can
        # start as soon as ~130KB (not 512KB) has landed
        for n in range(4):
            nsl = slice(n * 512, (n + 1) * 512)
            nc.sync.dma_start(xt[0][:, nsl], xT_d[0:128, nsl])
        wk01 = load_w(wk01_d, 128, "wk01")
        for n in range(4):
            nsl = slice(n * 512, (n + 1) * 512)
            nc.sync.dma_start(xt[1][:, nsl], xT_d[128:256, nsl])
        # later chunks + late-consumer weights go via the ACT hwdge queue —
        # a separate DMA ring, idle during the load phase — so they stream
        # in parallel with the sync-ring chunks instead of serializing
        # behind them (the serialized feed starved the Q chain and tripped
        # a mid-projection HAM re-throttle)
        for d in range(2, DCH):
            nc.scalar.dma_start(xt[d][:], xT_d[d * 128:(d + 1) * 128, :])
        wqk2 = load_w(wqk2_d, 128, "wqk2")
        wv = load_w(wv_d, 195, "wv")

        def load_small(dram, shape, dt, tag):
            t = persist.tile(shape, dt, tag=tag)
            nc.sync.dma_start(t[:], dram[:, :])
            return t

        bq01 = load_small(bq01_d, [128, 1], F32, "bq01")
        bk01 = load_small(bk01_d, [128, 1], F32, "bk01")
        bqk2 = load_small(bqk2_d, [128, 1], F32, "bqk2")
        bv = load_small(bv_d, [128, 195], F32, "bv")
        tri = load_small(tri_d, [128, 128], BF16, "tri")

        wo01 = persist.tile([128, D], BF16, tag="wo01")
        nc.sync.dma_start(wo01[:], wo01_d[:, :])
        wo2 = persist.tile([64, D], BF16, tag="wo2")
        nc.sync.dma_start(wo2[:], wo2_d[:, :])

        # ---- persistent intermediates ----
        QT01 = persist.tile([128, S], BF16, tag="QT01")
        KT01 = persist.tile([128, S], BF16, tag="KT01")
        QKT2 = persist.tile([128, S], BF16, tag="QKT2")
        # partition swap of QKT2: rows 0-63 = K2, rows 64-127 = Q2*s
        QK2s = persist.tile([128, S], BF16, tag="QK2s")
        # V augmented with a ones column per head ([V_h | 1] x 3, 195 cols per
        # s-tile) so the PV matmul's 65th output row is the softmax denominator
        v_sb = persist.tile([128, NKT * 195], BF16, tag="v_sb")
        zT01 = persist.tile([128, S], BF16, tag="zT01")
        zT2 = persist.tile([64, S], BF16, tag="zT2")

        # ---- QKV projections. PSUM bank layout keeps the V/QK2 chains OFF
        # banks 0-3: the attention score slots alias banks 0-3, so they must
        # only WAR-depend on the (early-released) Q/K chains — otherwise the
        # first scores serialize behind the whole V projection. ----
        with tc.tile_pool(name="proj_ps", bufs=4, space="PSUM") as proj_pool, \
             tc.tile_pool(name="qk2_ps", bufs=2, space="PSUM") as qk2_pool, \
             tc.tile_pool(name="v_ps", bufs=2, space="PSUM") as v_pool:
            NS = S // 512
            for w_s, bias_s, out_s, nm in ((wq01, bq01, QT01, "q"),
                                           (wk01, bk01, KT01, "k")):
                pss = [proj_pool.tile([128, 512], F32, tag="chain",
                                      name=f"{nm}{n}") for n in range(NS)]
                for d in range(DCH):
                    lhsT = w_s[:, d * 128:(d + 1) * 128]
                    for n in range(NS):
                        nc.tensor.matmul(pss[n][:], lhsT,
                                         xt[d][:, n * 512:(n + 1) * 512],
                                         start=(d == 0), stop=(d == DCH - 1))
                for n in range(NS):
                    nsl = slice(n * 512, (n + 1) * 512)
                    nc.vector.tensor_scalar_add(out_s[:, nsl], pss[n][:],
                                                bias_s[:])

            # QK2 chain through its own 2 banks (4-5)
            for n in range(NS):
                ps = qk2_pool.tile([128, 512], F32, tag="qk2", name=f"h{n}")
                for d in range(DCH):
                    nc.tensor.matmul(ps[:], wqk2[:, d * 128:(d + 1) * 128],
                                     xt[d][:, n * 512:(n + 1) * 512],
                                     start=(d == 0), stop=(d == DCH - 1))
                nsl = slice(n * 512, (n + 1) * 512)
                nc.vector.tensor_scalar_add(QKT2[:, nsl], ps[:], bqk2[:])

            # h2 partition swap (scores h2 packs two j-tiles per issue slot:
            # rows 0-63 use (K2, Q2) from (QK2s, QKT2), rows 64-127 from
            # (QKT2, QK2s))
            nc.sync.dma_start(QK2s[0:64, :], QKT2[64:128, :])
            nc.sync.dma_start(QK2s[64:128, :], QKT2[0:64, :])

            # V through banks 6-7
            for s_t in range(NKT):
                ps = v_pool.tile([128, 195], F32, tag="vps")
                for d in range(DCH):
                    nc.tensor.matmul(ps[:], xt[d][:, s_t * 128:(s_t + 1) * 128],
                                     wv[:, d * 195:(d + 1) * 195],
                                     start=(d == 0), stop=(d == DCH - 1))
                nc.vector.tensor_add(v_sb[:, s_t * 195:(s_t + 1) * 195], ps[:],
                                     bv[:])

        # ---- attention ----
        # PSUM budget (8 banks): sT (scores [128,1024]) 2 slots = 4 banks;
        # zts 3 x [65,512] = 3 banks; outproj 1 bank ([128,512] sub-tiles).
        with tc.tile_pool(name="sT_ps", bufs=2, space="PSUM") as sT_pool, \
             tc.tile_pool(name="zT_ps", bufs=3, space="PSUM") as zT_pool, \
             tc.tile_pool(name="op_ps", bufs=1, space="PSUM") as op_pool, \
             tc.tile_pool(name="pt_sb", bufs=10) as pt_pool, \
             tc.tile_pool(name="rb_sb", bufs=3) as rb_pool, \
             tc.tile_pool(name="zu_sb", bufs=4) as zu_pool, \
             tc.tile_pool(name="zs_sb", bufs=2) as zs_pool, \
             tc.tile_pool(name="out_sb", bufs=4) as out_pool, \
             tc.tile_pool(name="recip_dr", bufs=4, space="DRAM") as rdr_pool, \
             tc.tile_pool(name="recip_sb", bufs=6) as recip_pool:

            # per-qi zts tiles, per stream: zts[qi] = [zt_h0, zt_h1] for the
            # pair stream, zt2s[qi] for the h2 stream
            zts = {}
            zt2s = {}

            def exp_mask(qi, g, st, name):
                pt = pt_pool.tile([128, G * 512], BF16, tag="pt", name=name)
                r0 = g * G - 4 * qi
                s0 = r0 * 128 if r0 >= 0 else 0
                nc.scalar.activation(pt[:, s0:G * 512], st[:, s0:G * 512],
                                     AF.Exp)
                for jj in range(G):
                    r = (g * G + jj) - 4 * qi
                    off = jj * 512
                    if r >= 0:
                        if r > 0:
                            nc.vector.memset(pt[:, off:off + r * 128], 0.0)
                        dsl = slice(off + r * 128, off + (r + 1) * 128)
                        nc.vector.tensor_mul(pt[:, dsl], pt[:, dsl], tri[:])
                return pt

            def pv(qi, g, pt, hv):
                J = 4 * qi + 4
                zt = zts[qi][hv] if hv < 2 else zt2s[qi]
                for jj in range(G):
                    j = g * G + jj
                    nc.tensor.matmul(
                        zt[:],
                        v_sb[:, j * 195 + hv * 65:j * 195 + hv * 65 + 65],
                        pt[:, jj * 512:(jj + 1) * 512],
                        start=(j == 0), stop=(j == J - 1))

            # round list: pair rounds (h0+h1) run ahead of h2 rounds so the
            # ScalarE exp pipeline starts as soon as Q,K are projected; h2
            # and PV trail behind the QK2/V projections.
            rounds = []
            for qi in range(NQB):
                NG = (4 * qi + 4) // G
                rounds += [("pair", qi, g) for g in range(NG)]
                if qi >= 1:
                    NGp = (4 * (qi - 1) + 4) // G
                    rounds += [("h2", qi - 1, g) for g in range(NGp)]
            rounds += [("h2", NQB - 1, g) for g in range((4 * NQB) // G)]

            def evac_pair(qi):
                # evacuate z (bf16) + denominator rows to SBUF right after
                # the last PV: releases the PSUM zts tiles within a round,
                # so the next q-block's PV never waits on the 1/sums chain
                zu = zu_pool.tile([64, 2 * 512], BF16, tag="zu")
                s3 = recip_pool.tile([1, 2 * 512], F32, tag="s3")
                for h in range(2):
                    hsl = slice(h * 512, (h + 1) * 512)
                    nc.vector.tensor_copy(zu[:, hsl], zts[qi][h][0:64, :])
                    nc.vector.tensor_copy(s3[:, hsl], zts[qi][h][64:65, :])
                return zu, s3

            def reshape_sums(s3, n, tag, eng=None):
                # phase A of the 1/sums chain: denominator row -> DRAM ->
                # [128, n/128] partition reshape
                eng = eng or nc.sync
                dr1 = rdr_pool.tile([1, n], F32, tag="dr1" + tag)
                eng.dma_start(dr1[:], s3[:])
                rs = recip_pool.tile([128, n // 128], F32, tag="rs" + tag)
                eng.dma_start(
                    rs[:], dr1[:].rearrange("o (p f) -> (o p) f", p=128))
                return rs

            def bcast_recip(rs, n, tag, eng=None):
                # phase B (deferred so the DVE reciprocal never FIFO-waits
                # on phase A's DMAs): reciprocal -> DRAM -> partition
                # broadcast
                eng = eng or nc.sync
                rr = recip_pool.tile([128, n // 128], F32, tag="rr" + tag)
                nc.vector.reciprocal(rr[:], rs[:])
                dr2 = rdr_pool.tile([1, n], F32, tag="dr2" + tag)
                eng.dma_start(
                    dr2[:].rearrange("o (p f) -> (o p) f", p=128), rr[:])
                rb = rb_pool.tile([64, n], F32, tag="rb" + tag)
                eng.dma_start(rb[:], dr2[0:1, :].broadcast_to([64, n]))
                return rb

            def mul_pair(qi, zu, rb):
                # zT_h = zu_h * broadcast(1/sums_h); on GpSimd (SBUF-only
                # operands) and deferred far enough that no FIFO that gates
                # the PV/exp pipeline ever waits on the chain
                qsl = slice(qi * QB, qi * QB + QB)
                nc.gpsimd.tensor_mul(zT01[0:64, qsl], zu[:, 0:512],
                                     rb[:, 0:512])
                z1 = zs_pool.tile([64, 512], BF16, tag="z1")
                nc.gpsimd.tensor_mul(z1[:], zu[:, 512:1024], rb[:, 512:1024])
                # head 1 lives on partitions 64-127 of zT01: DMA partition-shift
                nc.sync.dma_start(zT01[64:128, qsl], z1[:])

            def evac_h2(qi):
                zu = zu_pool.tile([64, 512], BF16, tag="zu2")
                s3 = recip_pool.tile([1, 512], F32, tag="s1")
                nc.vector.tensor_copy(zu[:], zt2s[qi][0:64, :])
                nc.vector.tensor_copy(s3[:], zt2s[qi][64:65, :])
                return zu, s3

            def mul_h2(qi, zu, rb):
                qsl = slice(qi * QB, qi * QB + QB)
                nc.gpsimd.tensor_mul(zT2[:, qsl], zu[:], rb[:])

            obs = {}

            def outproj_sub(t, n0, pool=None, tag="op"):
                # one 256-wide slice of q-tile t's output projection.
                # q0-q2: both heads' matmuls accumulate in one PSUM tile,
                # single CAST out. q3 (the tail): h2 projects UNNORMALIZED
                # z into the second half-bank; DVE folds 1/sums_h2 in as a
                # per-partition scalar so the tail never waits a broadcast.
                pool = pool or op_pool
                qi, c = t // 4, t % 4
                tsl = slice(t * 128, (t + 1) * 128)
                if t not in obs:
                    obs[t] = out_pool.tile([128, D], BF16, tag="ob",
                                           name=f"ob{t}")
                ob = obs[t]
                ps = pool.tile([128, 256], F32, tag=tag, name=f"op{t}_{n0}")
                nc.tensor.matmul(ps[:], zT01[:, tsl], wo01[:, n0:n0 + 256],
                                 start=True, stop=False)
                nc.tensor.matmul(ps[:], zT2[:, tsl], wo2[:, n0:n0 + 256],
                                 start=False, stop=True)
                nc.vector.tensor_copy(ob[:, n0:n0 + 256], ps[:])
                if n0 + 256 == D:
                    # output DMA via the idle GpSimd queue: keeps the Sync
                    # FIFO free for the latency-sensitive 1/sums chain hops
                    nc.gpsimd.dma_start(out_d[tsl, :], ob[:])

            # software pipeline with one round of skew: round r's PV is
            # emitted after round r+1's score matmuls, so the PE always
            # has independent work while ACT runs exp. acts[] holds
            # (delay_in_rounds, fn) items: normalize muls and outproj halves
            # are deferred enough rounds that their inputs (DMA reciprocal
            # chains) are ready when they reach an engine FIFO — a premature
            # instruction in a strict-FIFO queue stalls everything behind it.
            pending = None
            acts = []
            opq = []
            hp = ctx.enter_context(tc.high_priority())

            def flush_acts():
                nonlocal acts
                rest = []
                for dly, fn in acts:
                    if dly <= 1:
                        fn()
                    else:
                        rest.append((dly - 1, fn))
                acts = rest
                n_drain = 2 if len(opq) > 6 else 1
                for _ in range(min(n_drain, len(opq))):
                    t, n0 = opq.pop(0)
                    outproj_sub(t, n0)

            for kind, qi, g in rounds:
                q0 = qi * QB
                qsl = slice(q0, q0 + QB)
                if kind == "pair":
                    if g == 0:
                        zts[qi] = [zT_pool.tile([65, 512], F32, tag="zT",
                                                name=f"zt{qi}h{h}")
                                   for h in range(2)]
                    st0 = sT_pool.tile([128, G * 512], F32, tag="sT",
                                       name="st0")
                    st1 = sT_pool.tile([128, G * 512], F32, tag="sT",
                                       name="st1")
                    # adjacent emission, alternating row groups -> the PE
                    # overlaps each (st0, st1) pair via row packing
                    for jj in range(G):
                        j = g * G + jj
                        ksl = slice(j * 128, (j + 1) * 128)
                        osl = slice(jj * 512, (jj + 1) * 512)
                        nc.tensor.matmul(st0[:, osl], KT01[0:64, ksl],
                                         QT01[0:64, qsl],
                                         start=True, stop=True)
                        nc.tensor.matmul(st1[:, osl], KT01[64:128, ksl],
                                         QT01[64:128, qsl],
                                         start=True, stop=True)
                    sts = [(st0, 0), (st1, 1)]
                else:
                    if g == 0:
                        zt2s[qi] = zT_pool.tile([65, 512], F32, tag="zT",
                                                name=f"zt{qi}h2")
                    st2 = sT_pool.tile([128, G * 512], F32, tag="sT",
                                       name="st2")
                    # jj0 on rows 0-63 (K2 in QK2s, Q2 in QKT2), jj1 on rows
                    # 64-127 (K2 in QKT2, Q2 in QK2s): packed pair
                    j0 = g * G
                    nc.tensor.matmul(st2[:, 0:512],
                                     QK2s[0:64, j0 * 128:(j0 + 1) * 128],
                                     QKT2[0:64, qsl], start=True, stop=True)
                    nc.tensor.matmul(st2[:, 512:1024],
                                     QKT2[64:128, (j0 + 1) * 128:(j0 + 2) * 128],
                                     QK2s[64:128, qsl], start=True, stop=True)
                    sts = [(st2, 2)]

                if pending is not None:
                    pq, pg, plist = pending
                    for ppt, phv in plist:
                        pv(pq, pg, ppt, phv)
                    flush_acts()
                pending = (qi, g, [(exp_mask(qi, g, st, f"pt{hv}"), hv)
                                   for st, hv in sts])

                J = 4 * qi + 4
                if kind == "pair" and g == J // G - 1:
                    def pair_done(q=qi):
                        zu, s3 = evac_pair(q)
                        rs = reshape_sums(s3, 1024, "p")

                        def phase_b():
                            rb = bcast_recip(rs, 1024, "p")
                            acts.append((3, lambda: mul_pair(q, zu, rb)))
                        acts.append((3, phase_b))
                    acts.append((1, pair_done))
                if kind == "h2" and g == J // G - 1:
                    def h2_done(q=qi):
                        # final q-block's chain hops go via the (by then
                        # idle) ACT hwdge queue: no sync-FIFO contention on
                        # the tail's critical path
                        eng = nc.scalar if q == NQB - 1 else nc.sync
                        zu, s3 = evac_h2(q)
                        rs = reshape_sums(s3, 512, "h", eng=eng)

                        def phase_b():
                            rb = bcast_recip(rs, 512, "h", eng=eng)
                            acts.append((3, lambda: mul_h2(q, zu, rb)))
                        acts.append((3, phase_b))
                        acts.append((9, lambda: opq.extend(
                            (t, n0) for t in range(4 * q, 4 * q + 4)
                            for n0 in (0, 256, 512))))
                    acts.append((1, h2_done))

            pq, pg, plist = pending
            for ppt, phv in plist:
                pv(pq, pg, ppt, phv)
            # drain remaining deferred work; alternate the leftover outproj
            # halves across the freed sT slots and the op bank for a short
            # tail
            i = 0
            while acts or opq:
                rest = []
                for dly, fn in acts:
                    if dly <= 1:
                        fn()
                    else:
                        rest.append((dly - 1, fn))
                acts = rest
                if opq:
                    t, n0 = opq.pop(0)
                    pool, tag = ((sT_pool, "sT") if i % 3 else (op_pool, "op"))
                    outproj_sub(t, n0, pool=pool, tag=tag)
                    i += 1

    nc.compile()
    return nc


def _get_nc():
    global _BUILT
    if _BUILT is None:
        _BUILT = build_nc()
    return _BUILT


def make_in_maps(inputs):
    x = np.asarray(inputs["normalized_resid_pre"], dtype=np.float32)
    W_Q = np.asarray(inputs["W_Q"], dtype=np.float32)
    W_K = np.asarray(inputs["W_K"], dtype=np.float32)
    W_V = np.asarray(inputs["W_V"], dtype=np.float32)
    W_O = np.asarray(inputs["W_O"], dtype=np.float32)
    b_Q = np.asarray(inputs["b_Q"], dtype=np.float32)
    b_K = np.asarray(inputs["b_K"], dtype=np.float32)
    b_V = np.asarray(inputs["b_V"], dtype=np.float32)
    sc = 1.0 / np.sqrt(np.float32(DH))

    in_maps = []
    for c in range(N_CORES):
        b = c // 4
        h = (c % 4) * 3
        hs = [h, h + 1, h + 2]
        m = {
            "xT": np.ascontiguousarray(x[b].T).astype(NPBF16),
            "wq01": np.concatenate([W_Q[hs[0]] * sc, W_Q[hs[1]] * sc],
                                   axis=1).astype(NPBF16),
            "wk01": np.concatenate([W_K[hs[0]], W_K[hs[1]]], axis=1).astype(NPBF16),
            "wqk2": np.concatenate([W_Q[hs[2]] * sc, W_K[hs[2]]],
                                   axis=1).astype(NPBF16),
            "wv": np.concatenate(
                sum(([W_V[hh], np.zeros((D, 1), np.float32)] for hh in hs), []),
                axis=1).astype(NPBF16),
            "wo01": np.concatenate([W_O[hs[0]], W_O[hs[1]]], axis=0).astype(NPBF16),
            "wo2": W_O[hs[2]].astype(NPBF16),
            "bq01": (np.concatenate([b_Q[hs[0]], b_Q[hs[1]]]) * sc)[:, None]
                    .astype(np.float32),
            "bk01": np.concatenate([b_K[hs[0]], b_K[hs[1]]])[:, None]
                    .astype(np.float32),
            "bqk2": np.concatenate([b_Q[hs[2]] * sc, b_K[hs[2]]])[:, None]
                    .astype(np.float32),
            "bv": np.ascontiguousarray(np.broadcast_to(
                np.concatenate(
                    sum(([b_V[hh], np.ones(1, np.float32)] for hh in hs), [])),
                (128, 195))).astype(np.float32),
        }
        in_maps.append(m)
    return in_maps


def kernel(**inputs):
    global LAST_EXEC_NS, LAST_RESULT
    nc = _get_nc()
    in_maps = make_in_maps(inputs)
    b_O = np.asarray(inputs["b_O"], dtype=np.float32)

    res = run_bass_kernel_spmd(nc, in_maps, core_ids=list(range(N_CORES)),
                               trace=TRACE_ENABLED)
    LAST_EXEC_NS = res.exec_time_ns
    LAST_RESULT = res
    parts = [np.asarray(r["out_p"], dtype=np.float32) for r in res.results]
    out0 = parts[0] + parts[1] + parts[2] + parts[3]
    out1 = parts[4] + parts[5] + parts[6] + parts[7]
    out = np.stack([out0, out1]) + b_O
    return out.astype(np.float32)
